# revision 1
# baseline (speedup 1.0000x reference)
"""HMM forward-algorithm (CgpHmm layer) Trainium2 Bass kernel.

Problem: B=64 sequences, T=2048 steps, S=512 hidden states, E=6 symbols.
  A  = softmax(A_kernel, axis=-1)   [S,S] row-stochastic transition
  Bm = softmax(B_kernel, axis=-1)   [S,E] emission
  I  = softmax(I_kernel)            [S]   initial
  loglik[b] = log sum_s (I * em_0) @ prod_t (A diag(em_t))   (scaled scan)

Strategy (8 NeuronCores, data-parallel over batch, 8 seqs/core):
  * State-major layout: y[s, b] as 4 chunks of [128, 8] in SBUF (bf16).
  * A stationary on the PE as 16 bf16 [128,128] tiles (FWL weight loads);
    per scan step 16 LDW+MM pairs accumulate y' = A^T-chunks @ y into PSUM.
  * Emission likelihood table em[s, t*8+b] (scaled by 6 to keep z~O(1))
    precomputed on-device into SBUF via one-hot matmuls from the PE
    (obs one-hot encoding is prepared host-side; it is a pure re-encoding
    of the integer observation tensor).
  * Per-step DVE multiply y'[chunk] = psum[chunk] * em[chunk] (bf16 out).
  * Every 16 steps: z = colsum(y) via ones-matmul, loglik += log z, and
    1/z is folded into the emission tile of step t+2 (deferred, off the
    critical path; scan itself never stalls on normalization).
  * Final: loglik += log(colsum(y_final)) - T*log(6).
"""

import math
import os
import time

import numpy as np
import ml_dtypes

import concourse.bass as bass
import concourse.bacc as bacc
import concourse.mybir as mybir
import concourse.tile as tile
from concourse import bass_utils

F32 = mybir.dt.float32
BF16 = mybir.dt.bfloat16

B, S, E = 64, 512, 6
NCORES = 8
BS = B // NCORES            # 8 sequences per core
C = S // 128                # 4 state chunks
T_FULL = 2048
NP = 16                     # normalization period

Exp = mybir.ActivationFunctionType.Exp
Ln = mybir.ActivationFunctionType.Ln
AX = mybir.AxisListType.X
MAX = mybir.AluOpType.max
ADD = mybir.AluOpType.add


def build_program(T=T_FULL, debug=False, reps=1, np_=NP, adt=BF16,
                  fused_dve=False, k_only=0, norm_ln=True, norm_fold=True):
    """Variants for benchmarking:
      np_       normalization period
      adt       dtype of stationary A tiles (and y/em when fp8)
      fused_dve single [128,4,512] PSUM tile + one DVE em-mul per step
      k_only    if >0, only that many k-chunks per m (WRONG numerics; timing)
    """
    ydt = adt if adt != BF16 else BF16
    nc = bacc.Bacc("TRN2", target_bir_lowering=False, debug=debug)

    A_k = nc.dram_tensor("A_kernel", [S, S], F32, kind="ExternalInput")
    B_k = nc.dram_tensor("B_kernel", [S, E], F32, kind="ExternalInput")
    I_k = nc.dram_tensor("I_kernel", [S], F32, kind="ExternalInput")
    OH = nc.dram_tensor("oh", [E, T * BS], BF16, kind="ExternalInput")
    CH = nc.dram_tensor("chain", [1, BS], F32, kind="ExternalInput")
    OUT = nc.dram_tensor("out", [1, BS], F32, kind="ExternalOutput")

    # norm events: t in {np-1, 2np-1, ...}, correction applied at t+2 <= T-1
    norm_steps = set(t for t in range(np_ - 1, T - 1, np_) if t + 2 <= T - 1)

    with tile.TileContext(nc) as tc:
        with (
            tc.tile_pool(name="singles", bufs=1) as singles,
            tc.tile_pool(name="work", bufs=2) as work,
            tc.tile_pool(name="small", bufs=2) as small,
            tc.tile_pool(name="ypool", bufs=2) as ypool,
            tc.tile_pool(name="em2pool", bufs=2) as em2pool,
            tc.tile_pool(name="ppre", bufs=2, space="PSUM") as ppre,
            tc.tile_pool(name="gpsum", bufs=1 if fused_dve else 4,
                         space="PSUM") as gpsum,
            tc.tile_pool(name="zpool", bufs=1, space="PSUM") as zpool,
            tc.tile_pool(name="bpool", bufs=1, space="PSUM") as bpool,
        ):
            # ---------------- load inputs ----------------
            a_in = []
            A_view = A_k[:].rearrange("(c p) s -> c p s", p=128)
            for k in range(C):
                t_ = work.tile([128, S], F32, tag=f"a_in{k}")
                nc.sync.dma_start(out=t_[:], in_=A_view[k])
                a_in.append(t_)

            bkT = singles.tile([E, S], F32, tag="bkT")
            nc.sync.dma_start(out=bkT[:], in_=B_k[:].rearrange("s e -> e s"))

            i_row = singles.tile([1, S], F32, tag="i_row")
            nc.sync.dma_start(out=i_row[:], in_=I_k[:].rearrange("(a s) -> a s", a=1))

            oh_sb = singles.tile([E, T * BS], BF16, tag="oh")
            nc.sync.dma_start(out=oh_sb[:], in_=OH[:])

            # ---------------- A = softmax rows -> bf16 chunks ----------------
            a_sb = []
            for k in range(C):
                negmax = small.tile([128, 1], F32, tag="negmax")
                nc.vector.tensor_reduce(negmax[:], a_in[k][:], axis=AX, op=MAX,
                                        negate=True)
                expd = work.tile([128, S], F32, tag="expd")
                nc.scalar.activation(expd[:], a_in[k][:], Exp, bias=negmax[:, 0:1])
                ssum = small.tile([128, 1], F32, tag="ssum")
                nc.vector.tensor_reduce(ssum[:], expd[:], axis=AX, op=ADD)
                sinv = small.tile([128, 1], F32, tag="sinv")
                nc.vector.reciprocal(sinv[:], ssum[:])
                ab = singles.tile([128, S], adt, tag=f"a_sb{k}")
                nc.vector.tensor_scalar_mul(ab[:], expd[:], sinv[:, 0:1])
                a_sb.append(ab)

            # ---------------- BmT6 = 6 * softmax(B_kernel) transposed --------
            expT = singles.tile([E, S], F32, tag="expT")
            nc.scalar.activation(expT[:], bkT[:], Exp)
            ones6 = singles.tile([E, 1], F32, tag="ones6")
            nc.vector.memset(ones6[:], 1.0)
            denT = ppre.tile([1, S], F32, tag="ps")
            nc.tensor.matmul(denT[:], ones6[:], expT[:], start=True, stop=True)
            denrT = singles.tile([1, S], F32, tag="denrT")
            nc.vector.reciprocal(denrT[:], denT[:])
            nc.vector.tensor_scalar_mul(denrT[:], denrT[:], 6.0)
            denr6 = singles.tile([E, S], F32, tag="denr6")
            nc.gpsimd.partition_broadcast(denr6[:], denrT[:], channels=E)
            bmT6 = singles.tile([E, S], BF16, tag="bmT6")
            nc.vector.tensor_mul(bmT6[:], expT[:], denr6[:])

            # ---------------- I = softmax(I_kernel); BmI = BmT6 * I ----------
            iexp = singles.tile([1, S], F32, tag="iexp")
            nc.scalar.activation(iexp[:], i_row[:], Exp)
            isum = small.tile([1, 1], F32, tag="isum")
            nc.vector.tensor_reduce(isum[:], iexp[:], axis=AX, op=ADD)
            iinv = small.tile([1, 1], F32, tag="iinv")
            nc.vector.reciprocal(iinv[:], isum[:])
            inorm = singles.tile([1, S], F32, tag="inorm")
            nc.vector.tensor_scalar_mul(inorm[:], iexp[:], iinv[:, 0:1])
            i6 = singles.tile([E, S], F32, tag="i6")
            nc.gpsimd.partition_broadcast(i6[:], inorm[:], channels=E)
            denr6i = singles.tile([E, S], F32, tag="denr6i")
            nc.vector.tensor_mul(denr6i[:], denr6[:], i6[:])
            bmI = singles.tile([E, S], BF16, tag="bmI")
            nc.vector.tensor_mul(bmI[:], expT[:], denr6i[:])

            # ---------------- emission table (bf16, SBUF-resident) ----------
            em_sb = singles.tile([128, C, T * BS], ydt, tag="em")
            n_tb = (T * BS) // 512
            for m in range(C):
                lhs = bmT6[:, m * 128:(m + 1) * 128]
                for tb in range(n_tb):
                    ps = ppre.tile([128, 512], F32, tag="ps")
                    nc.tensor.matmul(ps[:], lhs, oh_sb[:, tb * 512:(tb + 1) * 512],
                                     start=True, stop=True)
                    dst = em_sb[:, m, tb * 512:(tb + 1) * 512]
                    if tb % 2 == 0:
                        nc.vector.tensor_copy(dst, ps[:])
                    else:
                        nc.scalar.copy(dst, ps[:])

            # ---------------- constants / state ----------------
            ones_col = singles.tile([128, 1], ydt, tag="ones_col")
            nc.vector.memset(ones_col[:], 1.0)
            ones_row = singles.tile([1, 128], F32, tag="ones_row")
            nc.vector.memset(ones_row[:], 1.0)
            loglik = singles.tile([1, BS], F32, tag="loglik")
            nc.vector.memset(loglik[:], 0.0)

            # ---------------- y0 = I * em_0 (via BmI one-hot matmul) --------
            gshape = [128, C, 512] if fused_dve else [128, BS]
            y_cur = ypool.tile([128, C, BS], ydt, tag="y")
            if fused_dve:
                ps0 = gpsum.tile(gshape, F32, tag="g", name="g")
                for m in range(C):
                    nc.tensor.matmul(ps0[:, m, 0:BS], bmI[:, m * 128:(m + 1) * 128],
                                     oh_sb[:, 0:BS], start=True, stop=True)
                nc.vector.tensor_copy(y_cur[:], ps0[:, :, 0:BS])
            else:
                for m in range(C):
                    ps0 = gpsum.tile(gshape, F32, tag="g", name="g")
                    nc.tensor.matmul(ps0[:], bmI[:, m * 128:(m + 1) * 128],
                                     oh_sb[:, 0:BS], start=True, stop=True)
                    nc.vector.tensor_copy(y_cur[:, m, :], ps0[:])

            # ---------------- the scan ----------------
            n_k = k_only or C
            em2_map = {}
            for t in [t for _ in range(reps) for t in range(1, T)]:
                y_next = ypool.tile([128, C, BS], ydt, tag="y")
                em_src = em2_map.pop(t, None)
                gf = gpsum.tile(gshape, F32, tag="g", name="g") if fused_dve else None
                for m in range(C):
                    g = gf[:, m, 0:BS] if fused_dve else \
                        gpsum.tile(gshape, F32, tag="g", name="g")[:]
                    for k in range(n_k):
                        nc.tensor.matmul(
                            g,
                            a_sb[k][:, m * 128:(m + 1) * 128],
                            y_cur[:, k, :],
                            start=(k == 0), stop=(k == n_k - 1),
                        )
                    if not fused_dve:
                        nc.vector.tensor_mul(
                            y_next[:, m, :], g,
                            em_sb[:, m, t * BS:(t + 1) * BS]
                            if em_src is None else em_src[:, m, :])
                if fused_dve:
                    nc.vector.tensor_mul(y_next[:], gf[:, :, 0:BS],
                                         em_sb[:, :, t * BS:(t + 1) * BS]
                                         if em_src is None else em_src[:])

                if t in norm_steps:
                    zp = zpool.tile([1, BS], F32, tag="z")
                    for k in range(C):
                        nc.tensor.matmul(zp[:], ones_col[:], y_next[:, k, :],
                                         start=(k == 0), stop=(k == C - 1))
                    if norm_ln:
                        zlog = small.tile([1, BS], F32, tag="zlog")
                        nc.scalar.activation(zlog[:], zp[:], Ln)
                        nc.vector.tensor_add(loglik[:], loglik[:], zlog[:])
                    zrec = small.tile([1, BS], F32, tag="zrec")
                    nc.vector.reciprocal(zrec[:], zp[:])
                    if norm_fold:
                        bp = bpool.tile([128, BS], F32, tag="bp")
                        nc.tensor.matmul(bp[:], ones_row[:], zrec[:],
                                         start=True, stop=True)
                        em2 = em2pool.tile([128, C, BS], ydt, tag="em2")
                        for m in range(C):
                            nc.vector.tensor_mul(
                                em2[:, m, :],
                                em_sb[:, m, (t + 2) * BS:(t + 3) * BS],
                                bp[:],
                            )
                        em2_map[t + 2] = em2
                    else:
                        # keep zrec alive so it isn't dead code
                        nc.vector.tensor_add(loglik[:], loglik[:], zrec[:])

                y_cur = y_next

            # ---------------- finalize ----------------
            zf = zpool.tile([1, BS], F32, tag="z")
            for k in range(C):
                nc.tensor.matmul(zf[:], ones_col[:], y_cur[:, k, :],
                                 start=(k == 0), stop=(k == C - 1))
            zflog = small.tile([1, BS], F32, tag="zlog")
            nc.scalar.activation(zflog[:], zf[:], Ln)
            nc.vector.tensor_add(loglik[:], loglik[:], zflog[:])
            nc.vector.tensor_scalar_add(loglik[:], loglik[:],
                                        -float(T) * math.log(6.0))

            # serialization token (for benchmarking N chained executions in
            # one dispatch): loglik += 0 * chain
            ch_sb = singles.tile([1, BS], F32, tag="ch")
            nc.sync.dma_start(out=ch_sb[:], in_=CH[:])
            chz = small.tile([1, BS], F32, tag="chz")
            nc.vector.tensor_scalar_mul(chz[:], ch_sb[:], 0.0)
            nc.vector.tensor_add(loglik[:], loglik[:], chz[:])

            nc.sync.dma_start(out=OUT[:], in_=loglik[:])

    nc.compile()
    return nc


def make_onehot(obs_shard: np.ndarray, T: int) -> np.ndarray:
    """obs_shard [BS, T] ints -> one-hot [E, T*BS] bf16 with oh[e, t*BS+b]."""
    oh = np.zeros((E, T * BS), dtype=ml_dtypes.bfloat16)
    obs = np.asarray(obs_shard).astype(np.int64)
    tb = np.arange(T)[None, :] * BS + np.arange(BS)[:, None]  # [BS, T]
    oh[obs.reshape(-1), tb.reshape(-1)] = 1.0
    return oh


_CACHED = {}


def _get_program(T):
    if T not in _CACHED:
        t0 = time.time()
        _CACHED[T] = build_program(T)
        print(f"[kernel] built bass program T={T} in {time.time()-t0:.1f}s",
              flush=True)
    return _CACHED[T]


def kernel(obs, A_kernel, B_kernel, I_kernel, _trace=False):
    obs = np.asarray(obs)
    A_kernel = np.asarray(A_kernel, dtype=np.float32)
    B_kernel = np.asarray(B_kernel, dtype=np.float32)
    I_kernel = np.asarray(I_kernel, dtype=np.float32)
    Bfull, T = obs.shape
    assert Bfull == B

    nc = _get_program(T)

    in_maps = []
    for c in range(NCORES):
        shard = obs[c * BS:(c + 1) * BS]
        in_maps.append({
            "A_kernel": A_kernel,
            "B_kernel": B_kernel,
            "I_kernel": I_kernel,
            "oh": make_onehot(shard, T),
            "chain": np.zeros((1, BS), np.float32),
        })

    kw = {"trace": True} if _trace else {}
    res = bass_utils.run_bass_kernel_spmd(
        nc, in_maps, core_ids=list(range(NCORES)), **kw,
    )
    out = np.concatenate([r["out"].reshape(BS) for r in res.results])
    kernel._last_result = res
    return out.astype(np.float32)

